# revision 1
# baseline (speedup 1.0000x reference)
"""Trainium2 Bass kernel for the MINE-style segment_reduce problem.

Computes, for the fixed problem size B=16384, L=512, HID=768, TRANS=128:

    mask   = target.astype(f32)                     # [B, L] of {0,1}
    counts = max(mask.sum(1), 1)
    lf     = (mask @ label_embed) / counts          # [B, HID]
    net(t) = MLP(concat(t @ W_text.T + b_text, lf @ W_label.T + b_label))
    out    = mean(softplus(net(text[perm]))) + mean(softplus(-net(text)))

Key algebraic folding (exact in real arithmetic): the first two linear
layers collapse into

    h1 = relu(text @ A_t.T + (mask @ LW2) / counts + c0)
    A_t = W0[:, :T] @ W_text            # [T, HID]
    LW2 = (label_embed @ W_label.T) @ W0[:, T:].T   # [L, T]
    c0  = b0 + W0[:, :T] @ b_text + W0[:, T:] @ b_label

so label_embed never needs to reach the device, and the per-sample
network is two small matmuls + relu + softplus.

Sharding: data-parallel over B across 8 NeuronCores (2048 rows each).
negative_text = text[perm] is realized host-side as a per-shard gather of
the (transposed, bf16-cast) text matrix. Each core returns the partial sum
of softplus terms over its rows; the host adds 8 scalars and divides by B.

Device layout is feature-major ("T layout", batch on the free dimension),
prepared host-side, so every matmul streams the batch through a stationary
weight tile. All heavy operands are bf16 (the 0/1 mask is exact in bf16);
accumulation is f32 in PSUM.
"""

import numpy as np
import ml_dtypes

B, L, HID, TRANS = 16384, 512, 768, 128
NCORES = 8
BS = B // NCORES          # 2048 rows per core
BT = 512                  # batch tile (free-dim columns per PSUM bank)
NT = BS // BT             # 4 tiles per core
HC = HID // 128           # 6 contraction chunks for text
LC = L // 128             # 4 contraction chunks for the mask

BF16 = ml_dtypes.bfloat16
FP8 = ml_dtypes.float8_e4m3

_CACHE = {}


def _split_sync_waits(nc, mybir, maxw_default=1, maxw_drain=1):
    """Walrus in this container rejects too many sync-waits per instruction
    ("Too many sync wait commands"). Hoist excess waits onto NoOps that
    precede the instruction on the same engine."""
    for f in nc.m.functions:
        for bb in f.blocks:
            new = []
            for inst in bb.instructions:
                maxw = maxw_drain if type(inst).__name__ in ("InstDrain", "InstNoOp") else maxw_default
                si = inst.sync_info
                if si is not None and si.on_wait is not None and len(si.on_wait) > maxw:
                    waits = list(si.on_wait)
                    head, rest = waits[:-maxw], waits[-maxw:]
                    for k in range(0, len(head), maxw_drain):
                        nop = mybir.InstNoOp(name=f"{inst.name}-w{k}", ins=[], outs=[])
                        nop.engine = inst.engine
                        nop.sync_info = mybir.SyncInfo(
                            on_wait=head[k : k + maxw_drain], on_update=[]
                        )
                        new.append(nop)
                    inst.sync_info = mybir.SyncInfo(
                        on_wait=rest, on_update=list(si.on_update or [])
                    )
                new.append(inst)
            bb.instructions = new


N_WARM = 4
WC8_C = HC + LC                     # packed fp8 weight chunks: atT | lw2
WC16_W = TRANS + 1                  # packed bf16 weight columns: w1T | w2T
FP_W = BS + 3                       # packed f32 row: cinv | (-b2, +b2) | 1.0


def _build(maxw_default=1):
    import concourse.bass as bass
    import concourse.mybir as mybir
    import concourse.tile as tile

    f32 = mybir.dt.float32
    bf16 = mybir.dt.bfloat16

    nc = bass.Bass("TRN2", target_bir_lowering=False, debug=False, num_devices=NCORES)

    fp8 = mybir.dt.float8e4
    xt_d = nc.declare_dram_parameter("xt", [128, HC // 2, 2 * BS], fp8, isOutput=False)
    xn_d = nc.declare_dram_parameter("xn", [128, HC // 2, 2 * BS], fp8, isOutput=False)
    mt_d = nc.declare_dram_parameter("mt", [128, LC // 2, 2 * BS], fp8, isOutput=False)
    wc8_d = nc.declare_dram_parameter("wc8", [128, WC8_C * TRANS], fp8, isOutput=False)
    wc16_d = nc.declare_dram_parameter("wc16", [128, WC16_W], bf16, isOutput=False)
    fp_d = nc.declare_dram_parameter("fp", [1, FP_W], f32, isOutput=False)
    cb_d = nc.declare_dram_parameter("cbv", [1, BS], f32, isOutput=False)
    c0b1_d = nc.declare_dram_parameter("c0b1", [TRANS, 5], f32, isOutput=False)
    out_d = nc.declare_dram_parameter("out", [1, 1], f32, isOutput=True)

    AF = mybir.ActivationFunctionType
    ALU = mybir.AluOpType

    with tile.TileContext(nc) as tc:
        with (
            tc.tile_pool(name="const", bufs=1) as cpool,
            tc.tile_pool(name="xload", bufs=NT + 3) as xpool,
            tc.tile_pool(name="mload", bufs=NT + 2) as mpool,
            tc.tile_pool(name="work", bufs=2) as wpool,
            tc.tile_pool(name="psum_u", bufs=3, space="PSUM") as pu,
            tc.tile_pool(name="psum_v", bufs=1, space="PSUM") as pv,
            tc.tile_pool(name="psum_h2", bufs=2, space="PSUM") as ph2,
            tc.tile_pool(name="psum_e", bufs=2, space="PSUM") as pe,
        ):
            # ---- constants: 3 packed DMAs + 1 broadcast, all on the (idle)
            # gpsimd SWDGE queue so the Sync HWDGE ring is free for bulk data
            wc8_sb = cpool.tile([128, WC8_C, TRANS], fp8, tag="wc8")
            nc.gpsimd.dma_start(wc8_sb[:], wc8_d.ap().rearrange("p (c m) -> p c m", m=TRANS))
            wc16_sb = cpool.tile([128, WC16_W], bf16, tag="wc16")
            nc.gpsimd.dma_start(wc16_sb[:], wc16_d[:, :])
            fp_sb = cpool.tile([1, FP_W], f32, tag="fp")
            nc.gpsimd.dma_start(fp_sb[:], fp_d[:, :])
            c0b1_sb = cpool.tile([TRANS, 5], f32, tag="c0b1")
            nc.gpsimd.dma_start(c0b1_sb[:], c0b1_d[:, :])
            cb_sb = cpool.tile([128, BS], f32, tag="cb")

            def atT2(c):  # fp8 chunk PAIR [128, 2, TRANS] for DoubleRow
                return wc8_sb[:, 2 * c : 2 * c + 2, :]

            def lw22(c):
                return wc8_sb[:, HC + 2 * c : HC + 2 * c + 2, :]

            w1T = wc16_sb[:, 0:TRANS]
            w2T = wc16_sb[:, TRANS : TRANS + 1]
            c0 = c0b1_sb[:, 0:1]
            b1 = c0b1_sb[:, 1:2]
            ones_col = c0b1_sb[:, 2:3]
            sp_sgn = c0b1_sb[:, 3:4]
            sp_bias = c0b1_sb[:, 4:5]

            # staging row for all 2*BS e-values: [joint block | marginal block]
            ecat_sb = cpool.tile([1, 2 * BS], f32, tag="ecat")

            # ---- PE pre-warm: dummy matmuls with no input deps keep the PE
            # HAM activity window busy while the first loads are in flight,
            # so real matmuls start at 2.4 GHz instead of 1.2 GHz.
            warm_sb = cpool.tile([128, BT], bf16, tag="warmsb")
            nc.vector.memset(warm_sb[:, :], 0)
            warm_ps = pu.tile([128, BT], f32, tag="u")
            for _ in range(N_WARM):
                nc.tensor.matmul(
                    warm_ps[:, :], warm_sb[:, :TRANS], warm_sb[:, :],
                    start=True, stop=True,
                )

            # ---- bulk loads on the Sync HWDGE ring ----
            # Tile 0 is loaded in per-chunk-pair pieces (with its 1/counts
            # slice interleaved) so the first matmuls and the vs-chain start
            # as early as possible; tiles 1..3 use one big DMA per stream.
            # The cb slices use a DRAM-side step-0 AP to broadcast the [1, n]
            # row across all 128 partitions (SWDGE descriptor-gen for that
            # pattern measured ~10us on gpsimd, so they ride the HWDGE ring).
            mt_aps, xt_aps, xn_aps, cb_t = [], [], [], []

            def pair_view(t):
                return t.rearrange("p (n j) -> p j n", j=2)

            sl2_0 = slice(0, 2 * BT)
            mtp0 = []
            for g in range(LC // 2):
                t = mpool.tile([128, 2 * BT], fp8, tag="mt")
                mtp0.append(t)
            xtp0, xnp0 = [], []
            for g in range(HC // 2):
                xt0g = xpool.tile([128, 2 * BT], fp8, tag="xt")
                xtp0.append(xt0g)
                xn0g = xpool.tile([128, 2 * BT], fp8, tag="xn")
                xnp0.append(xn0g)
            cb0 = mpool.tile([128, BT], f32, tag="cbt")

            nc.sync.dma_start(mtp0[0][:], mt_d[:, 0, sl2_0])
            nc.sync.dma_start(cb0[:], cb_d[:, 0:BT].broadcast_to([128, BT]))
            nc.sync.dma_start(xtp0[0][:], xt_d[:, 0, sl2_0])
            nc.sync.dma_start(mtp0[1][:], mt_d[:, 1, sl2_0])
            nc.sync.dma_start(xtp0[1][:], xt_d[:, 1, sl2_0])
            nc.sync.dma_start(xtp0[2][:], xt_d[:, 2, sl2_0])
            for g in range(HC // 2):
                nc.sync.dma_start(xnp0[g][:], xn_d[:, g, sl2_0])

            mt_aps.append([pair_view(mtp0[g][:, :]) for g in range(LC // 2)])
            xt_aps.append([pair_view(xtp0[g][:, :]) for g in range(HC // 2)])
            xn_aps.append([pair_view(xnp0[g][:, :]) for g in range(HC // 2)])
            cb_t.append(cb0)

            for i in range(1, NT):
                sl2 = slice(2 * i * BT, 2 * (i + 1) * BT)
                sl1 = slice(i * BT, (i + 1) * BT)
                mt_i = mpool.tile([128, LC // 2, 2 * BT], fp8, tag="mt")
                nc.sync.dma_start(mt_i[:], mt_d[:, :, sl2])
                cbt_i = mpool.tile([128, BT], f32, tag="cbt")
                nc.sync.dma_start(cbt_i[:], cb_d[:, sl1].broadcast_to([128, BT]))
                xt_i = xpool.tile([128, HC // 2, 2 * BT], fp8, tag="xt")
                nc.sync.dma_start(xt_i[:], xt_d[:, :, sl2])
                xn_i = xpool.tile([128, HC // 2, 2 * BT], fp8, tag="xn")
                nc.sync.dma_start(xn_i[:], xn_d[:, :, sl2])
                mt_aps.append([pair_view(mt_i[:, g, :]) for g in range(LC // 2)])
                xt_aps.append([pair_view(xt_i[:, g, :]) for g in range(HC // 2)])
                xn_aps.append([pair_view(xn_i[:, g, :]) for g in range(HC // 2)])
                cb_t.append(cbt_i)

            # ---- main loop over batch tiles ----
            for i in range(NT):
                # v = (mask @ LW2).T for this tile, then vs = v / counts
                v_ps = pv.tile([128, BT], f32, tag="v")
                for c in range(LC // 2):
                    nc.tensor.matmul(
                        v_ps[:, :],
                        lw22(c),
                        mt_aps[i][c],
                        start=(c == 0),
                        stop=(c == LC // 2 - 1),
                        perf_mode=mybir.MatmulPerfMode.DoubleRow,
                    )
                vs_sb = wpool.tile([128, BT], f32, tag="vs")
                nc.vector.tensor_mul(vs_sb[:, :], v_ps[:, :], cb_t[i][:, :])

                for s, x_aps in enumerate((xt_aps[i], xn_aps[i])):
                    u_ps = pu.tile([128, BT], f32, tag="u")
                    for c in range(HC // 2):
                        nc.tensor.matmul(
                            u_ps[:, :],
                            atT2(c),
                            x_aps[c],
                            start=(c == 0),
                            stop=(c == HC // 2 - 1),
                            perf_mode=mybir.MatmulPerfMode.DoubleRow,
                        )
                    # u += c0 + vs in place (one DVE op), then relu -> bf16 on ACT
                    nc.vector.scalar_tensor_tensor(
                        u_ps[:, :],
                        u_ps[:, :],
                        c0,
                        vs_sb[:, :],
                        op0=ALU.add,
                        op1=ALU.add,
                    )
                    h1_sb = wpool.tile([128, BT], bf16, tag="h1")
                    nc.scalar.activation(h1_sb[:, :], u_ps[:, :], AF.Relu)

                    h2_ps = ph2.tile([128, BT], f32, tag="h2")
                    nc.tensor.matmul(
                        h2_ps[:, :], w1T, h1_sb[:, :], start=True, stop=True
                    )
                    h2_sb = wpool.tile([128, BT], bf16, tag="h2s")
                    nc.scalar.activation(h2_sb[:, :], h2_ps[:, :], AF.Relu, bias=b1)

                    e_ps = pe.tile([1, BT], f32, tag="e")
                    nc.tensor.matmul(
                        e_ps[:, :], w2T, h2_sb[:, :], start=True, stop=True
                    )
                    # stage e into the packed row: joint -> cols [0, BS),
                    # marginal -> cols [BS, 2*BS)
                    off = s * BS + i * BT
                    nc.vector.tensor_copy(ecat_sb[:, off : off + BT], e_ps[:, :])

            # softplus over all 4096 e-values at once, 128-partition parallel.
            # Pack the [1, 4096] row into [128, 32]: partitions 0-63 hold the
            # joint block, 64-127 the marginal block, so sign and b2-bias are
            # per-partition vectors. softplus(x) = ln(1 + exp(x)).
            EPK = 2 * BS // 128
            epk_sb = cpool.tile([128, EPK], f32, tag="epk")
            nc.sync.dma_start(epk_sb[:, :], ecat_sb[:, :])
            nc.scalar.activation(epk_sb[:, :], epk_sb[:, :], AF.Exp,
                                 bias=sp_bias, scale=sp_sgn)
            acc2_sb = cpool.tile([128, 1], f32, tag="acc2")
            sp_sb = cpool.tile([128, EPK], f32, tag="spout")
            nc.scalar.activation(sp_sb[:, :], epk_sb[:, :], AF.Ln,
                                 bias=fp_sb[:, BS + 2 : BS + 3].broadcast_to([128, 1])
                                 if False else ones_col,
                                 accum_out=acc2_sb[:, :])
            res_ps = pe.tile([1, 1], f32, tag="e")
            nc.tensor.matmul(res_ps[:, :], acc2_sb[:, :], ones_col,
                             start=True, stop=True)
            res_sb = cpool.tile([1, 1], f32, tag="res")
            nc.vector.tensor_copy(res_sb[:, :], res_ps[:, :])
            nc.sync.dma_start(out_d[:, :], res_sb[:, :])

    _split_sync_waits(nc, mybir, maxw_default=maxw_default, maxw_drain=1)
    return nc


def _get_nc():
    if "nc" not in _CACHE:
        _CACHE["nc"] = _build()
    return _CACHE["nc"]


def _prep_inputs(text_embed, label_embed, target, perm,
                 W_text, b_text, W_label, b_label, W0, b0, W1, b1, W2, b2):
    f64 = np.float64
    W0t = W0[:, :TRANS].astype(f64)
    W0l = W0[:, TRANS:].astype(f64)
    A_t = W0t @ W_text.astype(f64)                                   # [T, HID]
    LW2 = (label_embed.astype(f64) @ W_label.T.astype(f64)) @ W0l.T  # [L, T]
    c0 = b0.astype(f64) + W0t @ b_text.astype(f64) + W0l @ b_label.astype(f64)

    # packed fp8 weights [128, (atT 6 | lw2 4) chunks x 128] and bf16 head weights
    atT_p = np.ascontiguousarray(A_t.T).reshape(HC, 128, TRANS).transpose(1, 0, 2).reshape(128, HID)
    lw2_p = np.ascontiguousarray(LW2).reshape(LC, 128, TRANS).transpose(1, 0, 2).reshape(128, L)
    wc8 = np.concatenate([atT_p, lw2_p], axis=1).astype(FP8)
    wc16 = np.concatenate(
        [W1.T.astype(np.float64), W2.T.reshape(TRANS, 1).astype(np.float64)],
        axis=1).astype(BF16)
    spsgn = np.where(np.arange(TRANS) < TRANS // 2, -1.0, 1.0)
    b2s = float(np.asarray(b2).reshape(-1)[0])
    c0b1 = np.stack([
        c0, b1.astype(np.float64), np.ones(TRANS), spsgn, spsgn * b2s,
    ], axis=1).astype(np.float32)
    b2val = float(np.asarray(b2).reshape(-1)[0])

    counts = np.maximum(target.sum(axis=1), 1).astype(np.float64)
    cinv = (1.0 / counts).astype(np.float32)                         # [B]

    text_T = np.ascontiguousarray(text_embed.T).astype(FP8)         # [HID, B]
    mask_T = np.ascontiguousarray(target.T.astype(np.float32)).astype(FP8)  # [L, B]
    perm = np.asarray(perm).astype(np.int64)

    def interleave(a):
        # [2G*128, N] -> [128, G, 2N] with fp8 k-chunk pairs adjacent per column
        g2, n = a.shape[0] // 256, a.shape[1]
        return np.ascontiguousarray(
            a.reshape(g2, 2, 128, n).transpose(2, 0, 3, 1).reshape(128, g2, 2 * n)
        )

    in_maps = []
    for k in range(NCORES):
        sl = slice(k * BS, (k + 1) * BS)
        cinv_k = cinv[sl]
        fp = np.concatenate([cinv_k, [-b2val, b2val, 1.0]]).astype(np.float32).reshape(1, FP_W)
        in_maps.append({
            "xt": interleave(text_T[:, sl]),
            "xn": interleave(text_T[:, perm[sl]]),
            "mt": interleave(mask_T[:, sl]),
            "wc8": wc8, "wc16": wc16, "fp": fp,
            "cbv": cinv_k.reshape(1, BS).copy(),
            "c0b1": c0b1,
        })
    return in_maps, b2val

def _run(in_maps, b2val, trace=False):
    from concourse.bass_utils import run_bass_kernel_spmd

    nc = _get_nc()
    res = run_bass_kernel_spmd(nc, in_maps, list(range(NCORES)), trace=trace)
    total = sum(float(res.results[k]["out"][0, 0]) for k in range(NCORES))
    return np.float32(total / B), res


def kernel(text_embed, label_embed, target, perm,
           W_text, b_text, W_label, b_label, W0, b0, W1, b1, W2, b2):
    in_maps, b2val = _prep_inputs(
        text_embed, label_embed, target, perm,
        W_text, b_text, W_label, b_label, W0, b0, W1, b1, W2, b2)
    out, _ = _run(in_maps, b2val)
    return out



# revision 3
# speedup vs baseline: 1.2445x; 1.2445x over previous
"""Trainium2 Bass kernel for the MINE-style segment_reduce problem.

Computes, for the fixed problem size B=16384, L=512, HID=768, TRANS=128:

    mask   = target.astype(f32)                     # [B, L] of {0,1}
    counts = max(mask.sum(1), 1)
    lf     = (mask @ label_embed) / counts          # [B, HID]
    net(t) = MLP(concat(t @ W_text.T + b_text, lf @ W_label.T + b_label))
    out    = mean(softplus(net(text[perm]))) + mean(softplus(-net(text)))

Algebraic folding (exact in real arithmetic): the first two linear layers
collapse into

    h1 = relu(text @ A_t.T + (mask @ LW2) / counts + c0)
    A_t = W0[:, :T] @ W_text            # [T, HID]
    LW2 = (label_embed @ W_label.T) @ W0[:, T:].T   # [L, T]
    c0  = b0 + W0[:, :T] @ b_text + W0[:, T:] @ b_label

Device-side simplifications vs the v1 kernel:
  * 1/counts is folded into the mask host-side (fp8), so the mask matmuls
    accumulate directly into the text matmuls' PSUM bank and the h1 relu
    is a single ACT op with bias=c0 — no cinv broadcast DMA, no DVE adds.
  * The marginal term for text row g pairs text[g] with lf[ipos[g]]
    (ipos = perm^-1), so each core's negatives reuse its OWN text columns:
    no shuffled-text load, just a second (gathered, scaled) mask.
  * The 2*BS e-values per core are DMA'd out raw; softplus + mean happen
    on the host in f64.

Sharding: data-parallel over B across 8 NeuronCores (2048 rows each).
Device layout is feature-major (batch on the free dimension). All heavy
operands are fp8 with DoubleRow matmuls; accumulation is f32 in PSUM.
"""

import numpy as np
import ml_dtypes

B, L, HID, TRANS = 16384, 512, 768, 128
NCORES = 8
BS = B // NCORES          # 2048 rows per core
BT = 512                  # batch tile (free-dim columns per PSUM bank)
NT = BS // BT             # 4 tiles per core
HC = HID // 128           # 6 contraction chunks for text
LC = L // 128             # 4 contraction chunks for the mask

BF16 = ml_dtypes.bfloat16
FP8 = ml_dtypes.float8_e4m3

_CACHE = {}


def _split_sync_waits(nc, mybir, maxw_default=1, maxw_drain=1):
    """Walrus in this container rejects too many sync-waits per instruction
    ("Too many sync wait commands"). Hoist excess waits onto NoOps that
    precede the instruction on the same engine."""
    for f in nc.m.functions:
        for bb in f.blocks:
            new = []
            for inst in bb.instructions:
                maxw = maxw_drain if type(inst).__name__ in ("InstDrain", "InstNoOp") else maxw_default
                si = inst.sync_info
                if si is not None and si.on_wait is not None and len(si.on_wait) > maxw:
                    waits = list(si.on_wait)
                    head, rest = waits[:-maxw], waits[-maxw:]
                    for k in range(0, len(head), maxw_drain):
                        nop = mybir.InstNoOp(name=f"{inst.name}-w{k}", ins=[], outs=[])
                        nop.engine = inst.engine
                        nop.sync_info = mybir.SyncInfo(
                            on_wait=head[k : k + maxw_drain], on_update=[]
                        )
                        new.append(nop)
                    inst.sync_info = mybir.SyncInfo(
                        on_wait=rest, on_update=list(si.on_update or [])
                    )
                new.append(inst)
            bb.instructions = new


N_WARM = 4
WC8_C = HC + LC                     # packed fp8 weight chunks: atT | lw2
WC16_W = TRANS + 1                  # packed bf16 weight columns: w1T | w2T
TW = (HC + 2 * LC) * 512            # 7168 fp8 bytes/partition per tile: xt|mt|mtp
XW = HC * 512                       # 3072: xt block width within a tile row


def _build(maxw_default=1):
    import concourse.bass as bass
    import concourse.mybir as mybir
    import concourse.tile as tile

    f32 = mybir.dt.float32
    bf16 = mybir.dt.bfloat16
    fp8 = mybir.dt.float8e4

    nc = bass.Bass("TRN2", target_bir_lowering=False, debug=False, num_devices=NCORES)

    data_d = nc.declare_dram_parameter("data", [128, NT, TW], fp8, isOutput=False)
    wc8_d = nc.declare_dram_parameter("wc8", [128, WC8_C * TRANS], fp8, isOutput=False)
    wc16_d = nc.declare_dram_parameter("wc16", [128, WC16_W], bf16, isOutput=False)
    cb_d = nc.declare_dram_parameter("cvec", [TRANS, 2], f32, isOutput=False)
    out_d = nc.declare_dram_parameter("out", [1, 2 * BS], f32, isOutput=True)

    AF = mybir.ActivationFunctionType

    with tile.TileContext(nc) as tc:
        with (
            tc.tile_pool(name="const", bufs=1) as cpool,
            tc.tile_pool(name="dload", bufs=NT + 1) as dpool,
            tc.tile_pool(name="work", bufs=3) as wpool,
            tc.tile_pool(name="psum_u", bufs=2, space="PSUM") as pu,
            tc.tile_pool(name="psum_h2", bufs=2, space="PSUM") as ph2,
            tc.tile_pool(name="psum_e", bufs=2, space="PSUM") as pe,
        ):
            # ---- constants on the (otherwise idle) gpsimd SWDGE queue so
            # the Sync HWDGE ring is free for the bulk data loads
            wc8_sb = cpool.tile([128, WC8_C, TRANS], fp8, tag="wc8")
            nc.gpsimd.dma_start(wc8_sb[:], wc8_d.ap().rearrange("p (c m) -> p c m", m=TRANS))
            wc16_sb = cpool.tile([128, WC16_W], bf16, tag="wc16")
            nc.gpsimd.dma_start(wc16_sb[:], wc16_d[:, :])
            cvec_sb = cpool.tile([TRANS, 2], f32, tag="cvec")
            nc.gpsimd.dma_start(cvec_sb[:], cb_d[:, :])

            def atT2(c):  # fp8 chunk PAIR [128, 2, TRANS] for DoubleRow
                return wc8_sb[:, 2 * c : 2 * c + 2, :]

            def lw22(c):
                return wc8_sb[:, HC + 2 * c : HC + 2 * c + 2, :]

            w1T = wc16_sb[:, 0:TRANS]
            w2T = wc16_sb[:, TRANS : TRANS + 1]
            c0 = cvec_sb[:, 0:1]
            b1 = cvec_sb[:, 1:2]

            # staging row for all 2*BS e-values: [joint block | marginal block]
            ecat_sb = cpool.tile([1, 2 * BS], f32, tag="ecat")

            # ---- PE pre-warm: dummy matmuls with no input deps keep the PE
            # HAM activity window busy while the first loads are in flight.
            warm_sb = cpool.tile([128, BT], bf16, tag="warmsb")
            nc.vector.memset(warm_sb[:, :], 0)
            for _ in range(N_WARM):
                warm_ps = pu.tile([128, BT], f32, tag="u")
                nc.tensor.matmul(
                    warm_ps[:, :], warm_sb[:, :TRANS], warm_sb[:, :],
                    start=True, stop=True,
                )

            # ---- bulk loads on the Sync HWDGE ring: per-tile merged blocks,
            # fully contiguous per partition. Tile 0 is split so the joint
            # matmuls can start before its marginal mask arrives.
            def pair_view(t):  # [128, 1024] -> [128, 2, 512] DoubleRow pairs
                return t.rearrange("p (n j) -> p j n", j=2)

            d_t = []
            t0a = dpool.tile([128, HC // 2 + LC // 2, 512 * 2], fp8, tag="da")
            nc.sync.dma_start(
                t0a[:], data_d[:, 0, : XW + 1024 * (LC // 2)].rearrange(
                    "p (c m) -> p c m", m=1024))
            t0b = dpool.tile([128, LC // 2, 512 * 2], fp8, tag="db")
            nc.sync.dma_start(
                t0b[:], data_d[:, 0, XW + 1024 * (LC // 2):].rearrange(
                    "p (c m) -> p c m", m=1024))
            for i in range(1, NT):
                ti = dpool.tile([128, TW // 1024, 512 * 2], fp8, tag="da")
                nc.sync.dma_start(
                    ti[:], data_d[:, i, :].rearrange("p (c m) -> p c m", m=1024))
                d_t.append(ti)

            def xt_pairs(i, g):
                t = t0a if i == 0 else d_t[i - 1]
                return pair_view(t[:, g, :])

            def m_pairs(i, s, c):   # s=0 joint mask, s=1 marginal mask
                if i == 0:
                    t, base = (t0a, HC // 2) if s == 0 else (t0b, 0)
                else:
                    t, base = d_t[i - 1], HC // 2 + s * (LC // 2)
                return pair_view(t[:, base + c, :])

            # ---- main loop over batch tiles ----
            for i in range(NT):
                for s in range(2):  # 0 = joint, 1 = marginal
                    u_ps = pu.tile([128, BT], f32, tag="u")
                    for c in range(HC // 2):
                        nc.tensor.matmul(
                            u_ps[:, :], atT2(c), xt_pairs(i, c),
                            start=(c == 0), stop=False,
                            perf_mode=mybir.MatmulPerfMode.DoubleRow,
                        )
                    for c in range(LC // 2):
                        nc.tensor.matmul(
                            u_ps[:, :], lw22(c), m_pairs(i, s, c),
                            start=False, stop=(c == LC // 2 - 1),
                            perf_mode=mybir.MatmulPerfMode.DoubleRow,
                        )
                    h1_sb = wpool.tile([128, BT], bf16, tag="h1")
                    nc.scalar.activation(h1_sb[:, :], u_ps[:, :], AF.Relu, bias=c0)

                    h2_ps = ph2.tile([128, BT], f32, tag="h2")
                    nc.tensor.matmul(
                        h2_ps[:, :], w1T, h1_sb[:, :], start=True, stop=True
                    )
                    h2_sb = wpool.tile([128, BT], bf16, tag="h2s")
                    nc.scalar.activation(h2_sb[:, :], h2_ps[:, :], AF.Relu, bias=b1)

                    e_ps = pe.tile([1, BT], f32, tag="e")
                    nc.tensor.matmul(
                        e_ps[:, :], w2T, h2_sb[:, :], start=True, stop=True
                    )
                    # stage e into the packed row: joint -> cols [0, BS),
                    # marginal -> cols [BS, 2*BS)
                    off = s * BS + i * BT
                    nc.vector.tensor_copy(ecat_sb[:, off : off + BT], e_ps[:, :])

            nc.sync.dma_start(out_d[:, :], ecat_sb[:, :])

    _split_sync_waits(nc, mybir, maxw_default=maxw_default, maxw_drain=1)
    return nc


def _get_nc():
    if "nc" not in _CACHE:
        _CACHE["nc"] = _build()
    return _CACHE["nc"]


def _prep_inputs(text_embed, label_embed, target, perm,
                 W_text, b_text, W_label, b_label, W0, b0, W1, b1, W2, b2):
    f64 = np.float64
    W0t = W0[:, :TRANS].astype(f64)
    W0l = W0[:, TRANS:].astype(f64)
    A_t = W0t @ W_text.astype(f64)                                   # [T, HID]
    LW2 = (label_embed.astype(f64) @ W_label.T.astype(f64)) @ W0l.T  # [L, T]
    c0 = b0.astype(f64) + W0t @ b_text.astype(f64) + W0l @ b_label.astype(f64)

    # packed fp8 weights [128, (atT 6 | lw2 4) chunks x 128] and bf16 head weights
    atT_p = np.ascontiguousarray(A_t.T).reshape(HC, 128, TRANS).transpose(1, 0, 2).reshape(128, HID)
    lw2_p = np.ascontiguousarray(LW2).reshape(LC, 128, TRANS).transpose(1, 0, 2).reshape(128, L)
    wc8 = np.concatenate([atT_p, lw2_p], axis=1).astype(FP8)
    wc16 = np.concatenate(
        [W1.T.astype(f64), W2.T.reshape(TRANS, 1).astype(f64)],
        axis=1).astype(BF16)
    cvec = np.stack([c0, b1.astype(f64)], axis=1).astype(np.float32)
    b2val = float(np.asarray(b2).reshape(-1)[0])

    counts = np.maximum(target.sum(axis=1), 1).astype(f64)
    cinv = (1.0 / counts).astype(np.float32)                         # [B]
    perm = np.asarray(perm).astype(np.int64)
    ipos = np.argsort(perm)                                          # perm[ipos[g]] = g

    text_T = np.ascontiguousarray(text_embed.T).astype(FP8)          # [HID, B]
    msk = target.T.astype(np.float32) * cinv[None, :]                # [L, B] scaled
    mt8 = msk.astype(FP8)
    mtp8 = np.ascontiguousarray(msk[:, ipos]).astype(FP8)            # col g -> mask ipos[g]

    def interleave(a):
        # [2G*128, N] -> [128, G, 2N] with fp8 k-chunk pairs adjacent per column
        g2, n = a.shape[0] // 256, a.shape[1]
        return np.ascontiguousarray(
            a.reshape(g2, 2, 128, n).transpose(2, 0, 3, 1).reshape(128, g2, 2 * n)
        )

    in_maps = []
    for k in range(NCORES):
        sl = slice(k * BS, (k + 1) * BS)
        xt_i = interleave(text_T[:, sl])      # [128, 3, 2*BS]
        mt_i = interleave(mt8[:, sl])         # [128, 2, 2*BS]
        mtp_i = interleave(mtp8[:, sl])       # [128, 2, 2*BS]
        data = np.empty((128, NT, TW), dtype=FP8)
        for i in range(NT):
            sl2 = slice(2 * i * BT, 2 * (i + 1) * BT)
            data[:, i, :XW] = xt_i[:, :, sl2].reshape(128, XW)
            data[:, i, XW : XW + 2048] = mt_i[:, :, sl2].reshape(128, 2048)
            data[:, i, XW + 2048 :] = mtp_i[:, :, sl2].reshape(128, 2048)
        in_maps.append({"data": data, "wc8": wc8, "wc16": wc16, "cvec": cvec})
    return in_maps, b2val


def _run(in_maps, b2val, trace=False):
    from concourse.bass_utils import run_bass_kernel_spmd

    nc = _get_nc()
    res = run_bass_kernel_spmd(nc, in_maps, list(range(NCORES)), trace=trace)
    f64 = np.float64
    tot = 0.0
    for k in range(NCORES):
        e = np.asarray(res.results[k]["out"]).reshape(2 * BS).astype(f64) + b2val
        ej, em = e[:BS], e[BS:]
        sp = lambda x: np.log1p(np.exp(-np.abs(x))) + np.maximum(x, 0)
        tot += sp(em).sum() + sp(-ej).sum()
    return np.float32(tot / B), res


def kernel(text_embed, label_embed, target, perm,
           W_text, b_text, W_label, b_label, W0, b0, W1, b1, W2, b2):
    in_maps, b2val = _prep_inputs(
        text_embed, label_embed, target, perm,
        W_text, b_text, W_label, b_label, W0, b0, W1, b1, W2, b2)
    out, _ = _run(in_maps, b2val)
    return out
